# revision 38
# baseline (speedup 1.0000x reference)
"""Multi-head attention Bass kernel for Trainium2, sharded over 8 NeuronCores.

Problem: x [2, 2048, 1024] fp32; W_qkv [3072, 1024]; b_qkv [3072].
  qkv = x @ W_qkv.T + b_qkv ; split into Q,K,V of 8 heads x 128 dims;
  out  = softmax(Q K^T / sqrt(128)) V, heads re-concatenated -> [2, 2048, 1024].

Sharding: 16 (batch, head) pairs over 8 cores -> each core owns one batch
slice (b = core//4) and two heads (h0 = 2*(core%4), h0+1). Each core gets
its batch's x slice [2048, 1024] plus the W^T/bias columns for its heads,
computes the projection and full non-causal attention for its two heads,
and returns [2048, 256] (the two heads' output columns). No collectives.

Kernel structure (per core), tuned to keep the PE continuously busy:
 - K bias is dropped entirely: softmax_k[(Q+bq)·(K+bk)] == softmax_k[(Q+bq)·K]
   since the bk term is constant per query. V bias is added on the host:
   out = (P@V)/den + bv (softmax weights sum to 1).
 - All device inputs are host-packed so every DMA reads a single contiguous
   2-8KB segment per partition (max descriptor efficiency). x chunks are
   ragged (128,128,256,256,256,512,512 tokens) and split across two DGE
   queues (SP + DVE); weight pieces stream on the Act engine's queue in
   first-use order.
 - Projection phase interleaves K0 / V / Q0 per chunk. Q^T/K^T produced
   in [dh, tok] layout directly from the PE; V in [tok, dh] layout with a
   ones column so the P@V matmul also produces softmax denominators.
 - Attention is software-pipelined: score matmuls for group g+2 are
   emitted before the P@V matmuls of group g, so the PE never waits on
   the Act engine's exp. Remaining projection work (Q0 tail, K1, Q1) is
   interleaved as filler so the PE stays saturated while exp runs.
 - Epilogue per 128-token block: reciprocal + scale on DVE, bf16 out.
"""

import math
from contextlib import ExitStack

import numpy as np

import concourse.bass as bass
import concourse.tile as tile
from concourse import bacc, mybir
from concourse.bass_utils import run_bass_kernel_spmd

# Problem constants (hardcoded per the harness contract).
B = 2
S = 2048
D = 1024
H = 8
DH = 128
N_CORES = 8
HPC = 2  # heads per core
SC = S  # tokens per core (one full batch element)
SCALE = 1.0 / math.sqrt(DH)

F32 = mybir.dt.float32
BF16 = mybir.dt.bfloat16

KO = D // 128  # 8 contraction chunks
QB = 256  # query block width
NQB = SC // QB  # 8
NKT = SC // 128  # 16 key tiles
KPS = 4  # key tiles per score group; exp runs on [128, KPS*QB]
NG = NKT // KPS  # 4 score/pv groups per query block

# Ragged token chunks: small ones first so compute starts early.
CH_LENS = [128, 128, 256, 256, 256, 512, 512]
CHUNKS = []
_o = 0
for _ln in CH_LENS:
    CHUNKS.append((_o, _ln))
    _o += _ln
NCH = len(CHUNKS)
# token-block (128 tokens) -> (chunk index, local block index)
TB2C = []
for _ci, (_off, _ln) in enumerate(CHUNKS):
    for _lo in range(_ln // 128):
        TB2C.append((_ci, _lo))


def _mha_body(ctx: ExitStack, tc: tile.TileContext, out, x, wk0, wv, wq0, wk1, wq1, bias):
    nc = tc.nc

    consts = ctx.enter_context(tc.tile_pool(name="consts", bufs=1))
    xtp = ctx.enter_context(tc.tile_pool(name="xtp", bufs=1))
    qkvp = ctx.enter_context(tc.tile_pool(name="qkvp", bufs=1))

    # ---- input DMAs, ordered by first use ----
    # Three DGE queues: SP takes even x chunks; Act interleaves the early
    # weight pieces with odd x chunks; gpsimd (software DGE) takes the
    # late-needed K1/Q1 weights. All reads are contiguous per partition.
    xt = []
    xbase = []
    base = 0
    for c, (off, ln) in enumerate(CHUNKS):
        xt.append(xtp.tile([128, KO, ln], BF16, tag=f"xt{c}", name=f"xt{c}"))
        xbase.append(base)
        base += KO * ln

    def xsrc(c):
        off, ln = CHUNKS[c]
        return x[:, xbase[c]:xbase[c] + KO * ln].rearrange("p (ko t) -> p ko t", ko=KO)

    wtk, wtq = {}, {}
    wtk[0] = consts.tile([128, KO, DH], BF16, tag="wk0", name="wk0")
    wtv = consts.tile([128, KO, HPC * DH], BF16, tag="wv", name="wv")
    wtq[0] = consts.tile([128, KO, DH], BF16, tag="wq0", name="wq0")
    wtk[1] = consts.tile([128, KO, DH], BF16, tag="wk1", name="wk1")
    wtq[1] = consts.tile([128, KO, DH], BF16, tag="wq1", name="wq1")

    # Q biases [128 dh, 1] per head (K bias cancels; V bias added on host).
    bq = []
    for h in range(HPC):
        bt = consts.tile([128, 1], F32, tag=f"bq{h}", name=f"bq{h}")
        bq.append(bt)

    nc.gpsimd.dma_start(bq[0], bias[0:DH].rearrange("(p o) -> p o", o=1))
    nc.gpsimd.dma_start(bq[1], bias[DH:2 * DH].rearrange("(p o) -> p o", o=1))
    nc.sync.dma_start(xt[0], xsrc(0))
    nc.scalar.dma_start(wtk[0], wk0.rearrange("p (ko c) -> p ko c", ko=KO))
    nc.sync.dma_start(xt[1], xsrc(1))
    nc.scalar.dma_start(wtv, wv.rearrange("p (ko c) -> p ko c", ko=KO))
    nc.sync.dma_start(xt[2], xsrc(2))
    nc.scalar.dma_start(xt[3], xsrc(3))
    nc.sync.dma_start(wtq[0], wq0.rearrange("p (ko c) -> p ko c", ko=KO))
    nc.gpsimd.dma_start(xt[4], xsrc(4))
    nc.scalar.dma_start(xt[5], xsrc(5))
    nc.sync.dma_start(xt[6], xsrc(6))
    nc.gpsimd.dma_start(wtk[1], wk1.rearrange("p (ko c) -> p ko c", ko=KO))
    nc.gpsimd.dma_start(wtq[1], wq1.rearrange("p (ko c) -> p ko c", ko=KO))

    # hoist the ACT exp table load to kernel start, under the input DMAs
    warm = consts.tile([128, 1], F32)
    nc.vector.memset(warm, 0.0)
    nc.scalar.activation(warm, warm, mybir.ActivationFunctionType.Exp)

    # PE warmup: throwaway matmuls on memset tiles fill the initial input-DMA
    # wait and ramp the tensor engine's clock before real work arrives.
    dm_a = consts.tile([128, 128], BF16, tag="dm_a", name="dm_a")
    nc.vector.memset(dm_a, 0.0)
    dm_b = consts.tile([128, 512], BF16, tag="dm_b", name="dm_b")
    nc.vector.memset(dm_b, 0.0)

    # ---- persistent QKV tiles ----
    qT = qkvp.tile([128, HPC, SC], BF16, tag="qT")  # [dh, h, tok]
    kT = qkvp.tile([128, HPC, SC], BF16, tag="kT")
    v_sb = qkvp.tile([128, HPC, NKT, DH + 1], BF16, tag="v")  # [tok_i, h, tok_o, dh+1]
    nc.vector.memset(v_sb[:, :, :, DH:DH + 1], 1.0)

    projp = ctx.enter_context(tc.tile_pool(name="projp", bufs=2, space="PSUM"))
    stp = ctx.enter_context(tc.tile_pool(name="stp", bufs=2, space="PSUM"))
    pvp = ctx.enter_context(tc.tile_pool(name="pvp", bufs=2, space="PSUM"))
    atp = ctx.enter_context(tc.tile_pool(name="atp", bufs=5))
    outp = ctx.enter_context(tc.tile_pool(name="outp", bufs=3))
    rcp = ctx.enter_context(tc.tile_pool(name="rcp", bufs=4))

    # ---- projection emitters ----
    def kq_chunk(h, kind, c):
        off, ln = CHUNKS[c]
        ps = projp.tile([128, 512], F32, tag="ps", name="ps")
        w = (wtk if kind == "k" else wtq)[h]
        for ko in range(KO):
            nc.tensor.matmul(
                ps[:, :ln],
                lhsT=w[:, ko, :],
                rhs=xt[c][:, ko, :ln],
                start=(ko == 0),
                stop=(ko == KO - 1),
            )
        if kind == "q":
            nc.vector.tensor_scalar_add(qT[:, h, off:off + ln], ps[:, :ln], bq[h])
        else:
            nc.vector.tensor_copy(kT[:, h, off:off + ln], ps[:, :ln])

    def v_tb(tb):
        c, lo = TB2C[tb]
        ps = projp.tile([128, 512], F32, tag="ps", name="ps")
        psv = ps[:, :HPC * DH]
        for ko in range(KO):
            nc.tensor.matmul(
                psv,
                lhsT=xt[c][:, ko, lo * 128:(lo + 1) * 128],
                rhs=wtv[:, ko, :],
                start=(ko == 0),
                stop=(ko == KO - 1),
            )
        nc.vector.tensor_copy(
            v_sb[:, :, tb, 0:DH], psv.rearrange("p (h d) -> p h d", h=HPC)
        )

    # ---- attention emitters (software-pipelined) ----
    at_tiles = {}
    pv_tiles = {}

    def emit_sg(h, k):
        qb, g = divmod(k, NG)
        st = stp.tile([128, KPS, QB], F32, tag="st", name="st")
        for i in range(KPS):
            kt = g * KPS + i
            nc.tensor.matmul(
                st[:, i, :],
                lhsT=kT[:, h, kt * 128:(kt + 1) * 128],
                rhs=qT[:, h, qb * QB:(qb + 1) * QB],
                start=True,
                stop=True,
            )
        at = atp.tile([128, KPS, QB], BF16, tag="at", name="at")
        nc.scalar.activation(at, st, mybir.ActivationFunctionType.Exp, scale=SCALE)
        at_tiles[(h, k)] = at

    def emit_pg(h, k):
        qb, g = divmod(k, NG)
        if g == 0:
            # separate tiles so each P@V accumulator gets its own PSUM bank
            pv_tiles[(h, qb)] = [
                pvp.tile([128, DH + 1], F32, tag="pv", name=f"pv{j}")
                for j in range(QB // 128)
            ]
        pv = pv_tiles[(h, qb)]
        at = at_tiles.pop((h, k))
        for i in range(KPS):
            kt = g * KPS + i
            for j in range(QB // 128):
                nc.tensor.matmul(
                    pv[j],
                    lhsT=at[:, i, j * 128:(j + 1) * 128],
                    rhs=v_sb[:, h, kt, :],
                    start=(kt == 0),
                    stop=(kt == NKT - 1),
                )

    def epilogue(h, qb):
        pv = pv_tiles.pop((h, qb))
        for j in range(QB // 128):
            rc = rcp.tile([128, 1], F32, tag="rc", name="rc")
            nc.vector.reciprocal(rc, pv[j][:, DH:DH + 1])
            ot = outp.tile([128, DH], BF16, tag="ot", name="ot")
            nc.vector.tensor_scalar_mul(ot, pv[j][:, 0:DH], rc)
            nc.sync.dma_start(
                out[qb * QB + j * 128:qb * QB + (j + 1) * 128, h * DH:(h + 1) * DH],
                ot,
            )

    def attn(h, fills, fill_slots):
        emit_sg(h, 0)
        emit_sg(h, 1)
        fi = 0
        for k in range(NQB * NG):
            if k + 2 < NQB * NG:
                emit_sg(h, k + 2)
            emit_pg(h, k)
            if fi < len(fills) and k >= fill_slots[fi]:
                fills[fi]()
                fi += 1
            if k % NG == NG - 1:
                epilogue(h, k // NG)
        while fi < len(fills):
            fills[fi]()
            fi += 1

    # ---- schedule ----
    # PE warmup (no data dependencies), then pre-phase: K0 | V | Q0
    # interleaved per chunk to match DMA arrival order.
    # Chunks: c0,c1=128tok; c2..c4=256tok; c5,c6=512tok.
    for _ in range(20):
        ps = projp.tile([128, 512], F32, tag="ps", name="ps")
        nc.tensor.matmul(ps, lhsT=dm_a, rhs=dm_b, start=True, stop=True)

    kq_chunk(0, "k", 0)
    kq_chunk(0, "k", 1)
    v_tb(0)
    v_tb(1)
    kq_chunk(0, "k", 2)
    v_tb(2); v_tb(3)
    kq_chunk(0, "q", 0)
    kq_chunk(0, "q", 1)
    kq_chunk(0, "q", 2)
    kq_chunk(0, "k", 3)
    v_tb(4); v_tb(5)
    kq_chunk(0, "q", 3)
    kq_chunk(0, "k", 4)
    v_tb(6); v_tb(7)
    kq_chunk(0, "q", 4)
    kq_chunk(0, "k", 5)
    v_tb(8); v_tb(9); v_tb(10); v_tb(11)
    kq_chunk(0, "k", 6)
    v_tb(12); v_tb(13); v_tb(14); v_tb(15)

    # attn0 fills: Q0 tail (qb4-7), all of K1, Q1 head (tokens 0:512).
    fills0 = [
        lambda: kq_chunk(0, "q", 5),
        lambda: kq_chunk(1, "k", 0),
        lambda: kq_chunk(1, "k", 1),
        lambda: kq_chunk(0, "q", 6),
        lambda: kq_chunk(1, "k", 2),
        lambda: kq_chunk(1, "k", 3),
        lambda: kq_chunk(1, "k", 4),
        lambda: kq_chunk(1, "k", 5),
        lambda: kq_chunk(1, "k", 6),
        lambda: kq_chunk(1, "q", 0),
        lambda: kq_chunk(1, "q", 1),
        lambda: kq_chunk(1, "q", 2),
    ]
    attn(0, fills0, [1, 3, 5, 7, 9, 11, 13, 15, 17, 19, 21, 23])

    # attn1 fills: Q1 tail (tokens 512:2048), needed from qb2 onward.
    fills1 = [
        lambda: kq_chunk(1, "q", 3),
        lambda: kq_chunk(1, "q", 4),
        lambda: kq_chunk(1, "q", 5),
        lambda: kq_chunk(1, "q", 6),
    ]
    attn(1, fills1, [0, 1, 2, 3])


def build_program():
    nc = bacc.Bacc("TRN2", target_bir_lowering=False, debug=False)
    x = nc.dram_tensor("x", [128, KO * SC], BF16, kind="ExternalInput").ap()
    wk0 = nc.dram_tensor("wk0", [128, KO * DH], BF16, kind="ExternalInput").ap()
    wv = nc.dram_tensor("wv", [128, KO * HPC * DH], BF16, kind="ExternalInput").ap()
    wq0 = nc.dram_tensor("wq0", [128, KO * DH], BF16, kind="ExternalInput").ap()
    wk1 = nc.dram_tensor("wk1", [128, KO * DH], BF16, kind="ExternalInput").ap()
    wq1 = nc.dram_tensor("wq1", [128, KO * DH], BF16, kind="ExternalInput").ap()
    bias = nc.dram_tensor("bias", [HPC * DH], F32, kind="ExternalInput").ap()
    out = nc.dram_tensor("out", [SC, HPC * DH], BF16, kind="ExternalOutput").ap()
    with tile.TileContext(nc) as tc:
        with ExitStack() as ctx:
            _mha_body(ctx, tc, out, x, wk0, wv, wq0, wk1, wq1, bias)
    nc.compile()
    return nc


_NC = None


def _get_nc():
    global _NC
    if _NC is None:
        _NC = build_program()
    return _NC


def _pack_w(Wc):
    """[1024 rows, cols] f32 -> [128 ki, KO*cols] bf16, ki-major contiguous."""
    import ml_dtypes

    cols = Wc.shape[1]
    w = Wc.reshape(KO, 128, cols).transpose(1, 0, 2).reshape(128, KO * cols)
    return np.ascontiguousarray(w.astype(ml_dtypes.bfloat16))


def make_in_maps(x, W_qkv, b_qkv):
    import ml_dtypes

    x = np.asarray(x, dtype=np.float32)
    W = np.asarray(W_qkv, dtype=np.float32)
    b = np.asarray(b_qkv, dtype=np.float32)
    x_bf = x.astype(ml_dtypes.bfloat16)
    in_maps = []
    for c in range(N_CORES):
        bsel = c // 4
        h0 = HPC * (c % 4)
        qr = lambda h: np.arange((h0 + h) * DH, (h0 + h + 1) * DH)
        # x^T [1024, 2048] -> [128 ki, KO, tok] -> chunk-packed [128, KO*2048]
        xT = x_bf[bsel].T.reshape(KO, 128, SC).transpose(1, 0, 2)  # [128, KO, 2048]
        xpk = np.concatenate(
            [xT[:, :, off:off + ln].reshape(128, KO * ln) for off, ln in CHUNKS],
            axis=1,
        )
        in_maps.append(
            {
                "x": np.ascontiguousarray(xpk),
                "wk0": _pack_w(W[D + qr(0)].T),
                "wv": _pack_w(W[2 * D:][np.concatenate([qr(0), qr(1)])].T),
                "wq0": _pack_w(W[qr(0)].T),
                "wk1": _pack_w(W[D + qr(1)].T),
                "wq1": _pack_w(W[qr(1)].T),
                "bias": np.ascontiguousarray(np.concatenate([b[qr(0)], b[qr(1)]])),
            }
        )
    return in_maps


def gather_output(results, b_qkv):
    outp = np.empty((B, S, D), np.float32)
    for c in range(N_CORES):
        o = np.asarray(results[c]["out"], dtype=np.float32)
        bsel = c // 4
        h0 = HPC * (c % 4)
        outp[bsel, :, h0 * DH:(h0 + HPC) * DH] = o
    # V bias: out = (P@V)/den + bv since softmax weights sum to 1.
    outp += np.asarray(b_qkv, dtype=np.float32)[2 * D:][None, None, :]
    return outp


def kernel(x, W_qkv, b_qkv, **run_kwargs):
    in_maps = make_in_maps(x, W_qkv, b_qkv)
    res = run_bass_kernel_spmd(_get_nc(), in_maps, core_ids=list(range(N_CORES)), **run_kwargs)
    out = gather_output(res.results, b_qkv)
    if run_kwargs:
        kernel.last_result = res
    return out


# revision 39
# speedup vs baseline: 1.0144x; 1.0144x over previous
"""Multi-head attention Bass kernel for Trainium2, sharded over 8 NeuronCores.

Problem: x [2, 2048, 1024] fp32; W_qkv [3072, 1024]; b_qkv [3072].
  qkv = x @ W_qkv.T + b_qkv ; split into Q,K,V of 8 heads x 128 dims;
  out  = softmax(Q K^T / sqrt(128)) V, heads re-concatenated -> [2, 2048, 1024].

Sharding: 16 (batch, head) pairs over 8 cores -> each core owns one batch
slice (b = core//4) and two heads (h0 = 2*(core%4), h0+1). Each core gets
its batch's x slice [2048, 1024] plus the W^T/bias columns for its heads,
computes the projection and full non-causal attention for its two heads,
and returns [2048, 256] (the two heads' output columns). No collectives.

Kernel structure (per core), tuned to keep the PE continuously busy:
 - K bias is dropped entirely: softmax_k[(Q+bq)·(K+bk)] == softmax_k[(Q+bq)·K]
   since the bk term is constant per query. V bias is added on the host:
   out = (P@V)/den + bv (softmax weights sum to 1).
 - All device inputs are host-packed so every DMA reads a single contiguous
   2-8KB segment per partition (max descriptor efficiency). x chunks are
   ragged (128,128,256,256,256,512,512 tokens) and split across two DGE
   queues (SP + DVE); weight pieces stream on the Act engine's queue in
   first-use order.
 - Projection phase interleaves K0 / V / Q0 per chunk. Q^T/K^T produced
   in [dh, tok] layout directly from the PE; V in [tok, dh] layout with a
   ones column so the P@V matmul also produces softmax denominators.
 - Attention is software-pipelined: score matmuls for group g+2 are
   emitted before the P@V matmuls of group g, so the PE never waits on
   the Act engine's exp. Remaining projection work (Q0 tail, K1, Q1) is
   interleaved as filler so the PE stays saturated while exp runs.
 - Epilogue per 128-token block: reciprocal + scale on DVE, bf16 out.
"""

import math
from contextlib import ExitStack

import numpy as np

import concourse.bass as bass
import concourse.tile as tile
from concourse import bacc, mybir
from concourse.bass_utils import run_bass_kernel_spmd

# Problem constants (hardcoded per the harness contract).
B = 2
S = 2048
D = 1024
H = 8
DH = 128
N_CORES = 8
HPC = 2  # heads per core
SC = S  # tokens per core (one full batch element)
SCALE = 1.0 / math.sqrt(DH)

F32 = mybir.dt.float32
BF16 = mybir.dt.bfloat16

KO = D // 128  # 8 contraction chunks
QB = 256  # query block width
NQB = SC // QB  # 8
NKT = SC // 128  # 16 key tiles
KPS = 4  # key tiles per score group; exp runs on [128, KPS*QB]
NG = NKT // KPS  # 4 score/pv groups per query block

# Ragged token chunks: small ones first so compute starts early.
CH_LENS = [128, 128, 256, 256, 256, 512, 512]
CHUNKS = []
_o = 0
for _ln in CH_LENS:
    CHUNKS.append((_o, _ln))
    _o += _ln
NCH = len(CHUNKS)
# token-block (128 tokens) -> (chunk index, local block index)
TB2C = []
for _ci, (_off, _ln) in enumerate(CHUNKS):
    for _lo in range(_ln // 128):
        TB2C.append((_ci, _lo))


def _mha_body(ctx: ExitStack, tc: tile.TileContext, out, x, wk0, wv, wq0, wk1, wq1, bias):
    nc = tc.nc

    consts = ctx.enter_context(tc.tile_pool(name="consts", bufs=1))
    xtp = ctx.enter_context(tc.tile_pool(name="xtp", bufs=1))
    qkvp = ctx.enter_context(tc.tile_pool(name="qkvp", bufs=1))

    # ---- input DMAs, ordered by first use ----
    # Three DGE queues: SP takes even x chunks; Act interleaves the early
    # weight pieces with odd x chunks; gpsimd (software DGE) takes the
    # late-needed K1/Q1 weights. All reads are contiguous per partition.
    xt = []
    xbase = []
    base = 0
    for c, (off, ln) in enumerate(CHUNKS):
        xt.append(xtp.tile([128, KO, ln], BF16, tag=f"xt{c}", name=f"xt{c}"))
        xbase.append(base)
        base += KO * ln

    def xsrc(c):
        off, ln = CHUNKS[c]
        return x[:, xbase[c]:xbase[c] + KO * ln].rearrange("p (ko t) -> p ko t", ko=KO)

    wtk, wtq = {}, {}
    wtk[0] = consts.tile([128, KO, DH], BF16, tag="wk0", name="wk0")
    wtv = consts.tile([128, KO, HPC * DH], BF16, tag="wv", name="wv")
    wtq[0] = consts.tile([128, KO, DH], BF16, tag="wq0", name="wq0")
    wtk[1] = consts.tile([128, KO, DH], BF16, tag="wk1", name="wk1")
    wtq[1] = consts.tile([128, KO, DH], BF16, tag="wq1", name="wq1")

    # Q biases [128 dh, 1] per head (K bias cancels; V bias added on host).
    bq = []
    for h in range(HPC):
        bt = consts.tile([128, 1], F32, tag=f"bq{h}", name=f"bq{h}")
        bq.append(bt)

    nc.gpsimd.dma_start(bq[0], bias[0:DH].rearrange("(p o) -> p o", o=1))
    nc.gpsimd.dma_start(bq[1], bias[DH:2 * DH].rearrange("(p o) -> p o", o=1))
    nc.sync.dma_start(xt[0], xsrc(0))
    nc.scalar.dma_start(wtk[0], wk0.rearrange("p (ko c) -> p ko c", ko=KO))
    nc.sync.dma_start(xt[1], xsrc(1))
    nc.scalar.dma_start(wtv, wv.rearrange("p (ko c) -> p ko c", ko=KO))
    nc.sync.dma_start(xt[2], xsrc(2))
    nc.scalar.dma_start(xt[3], xsrc(3))
    nc.sync.dma_start(wtq[0], wq0.rearrange("p (ko c) -> p ko c", ko=KO))
    nc.sync.dma_start(xt[4], xsrc(4))
    nc.scalar.dma_start(xt[5], xsrc(5))
    nc.sync.dma_start(xt[6], xsrc(6))
    nc.gpsimd.dma_start(wtk[1], wk1.rearrange("p (ko c) -> p ko c", ko=KO))
    nc.gpsimd.dma_start(wtq[1], wq1.rearrange("p (ko c) -> p ko c", ko=KO))

    # hoist the ACT exp table load to kernel start, under the input DMAs
    warm = consts.tile([128, 1], F32)
    nc.vector.memset(warm, 0.0)
    nc.scalar.activation(warm, warm, mybir.ActivationFunctionType.Exp)

    # PE warmup: throwaway matmuls on memset tiles fill the initial input-DMA
    # wait and ramp the tensor engine's clock before real work arrives.
    dm_a = consts.tile([128, 128], BF16, tag="dm_a", name="dm_a")
    nc.vector.memset(dm_a, 0.0)
    dm_b = consts.tile([128, 512], BF16, tag="dm_b", name="dm_b")
    nc.vector.memset(dm_b, 0.0)

    # ---- persistent QKV tiles ----
    qT = qkvp.tile([128, HPC, SC], BF16, tag="qT")  # [dh, h, tok]
    kT = qkvp.tile([128, HPC, SC], BF16, tag="kT")
    v_sb = qkvp.tile([128, HPC, NKT, DH + 1], BF16, tag="v")  # [tok_i, h, tok_o, dh+1]
    nc.vector.memset(v_sb[:, :, :, DH:DH + 1], 1.0)

    projp = ctx.enter_context(tc.tile_pool(name="projp", bufs=2, space="PSUM"))
    stp = ctx.enter_context(tc.tile_pool(name="stp", bufs=2, space="PSUM"))
    pvp = ctx.enter_context(tc.tile_pool(name="pvp", bufs=2, space="PSUM"))
    atp = ctx.enter_context(tc.tile_pool(name="atp", bufs=5))
    outp = ctx.enter_context(tc.tile_pool(name="outp", bufs=3))
    rcp = ctx.enter_context(tc.tile_pool(name="rcp", bufs=4))

    # ---- projection emitters ----
    def kq_chunk(h, kind, c):
        off, ln = CHUNKS[c]
        ps = projp.tile([128, 512], F32, tag="ps", name="ps")
        w = (wtk if kind == "k" else wtq)[h]
        for ko in range(KO):
            nc.tensor.matmul(
                ps[:, :ln],
                lhsT=w[:, ko, :],
                rhs=xt[c][:, ko, :ln],
                start=(ko == 0),
                stop=(ko == KO - 1),
            )
        if kind == "q":
            nc.vector.tensor_scalar_add(qT[:, h, off:off + ln], ps[:, :ln], bq[h])
        else:
            nc.vector.tensor_copy(kT[:, h, off:off + ln], ps[:, :ln])

    def v_tb(tb):
        c, lo = TB2C[tb]
        ps = projp.tile([128, 512], F32, tag="ps", name="ps")
        psv = ps[:, :HPC * DH]
        for ko in range(KO):
            nc.tensor.matmul(
                psv,
                lhsT=xt[c][:, ko, lo * 128:(lo + 1) * 128],
                rhs=wtv[:, ko, :],
                start=(ko == 0),
                stop=(ko == KO - 1),
            )
        nc.vector.tensor_copy(
            v_sb[:, :, tb, 0:DH], psv.rearrange("p (h d) -> p h d", h=HPC)
        )

    # ---- attention emitters (software-pipelined) ----
    at_tiles = {}
    pv_tiles = {}

    def emit_sg(h, k):
        qb, g = divmod(k, NG)
        st = stp.tile([128, KPS, QB], F32, tag="st", name="st")
        for i in range(KPS):
            kt = g * KPS + i
            nc.tensor.matmul(
                st[:, i, :],
                lhsT=kT[:, h, kt * 128:(kt + 1) * 128],
                rhs=qT[:, h, qb * QB:(qb + 1) * QB],
                start=True,
                stop=True,
            )
        at = atp.tile([128, KPS, QB], BF16, tag="at", name="at")
        nc.scalar.activation(at, st, mybir.ActivationFunctionType.Exp, scale=SCALE)
        at_tiles[(h, k)] = at

    def emit_pg(h, k):
        qb, g = divmod(k, NG)
        if g == 0:
            # separate tiles so each P@V accumulator gets its own PSUM bank
            pv_tiles[(h, qb)] = [
                pvp.tile([128, DH + 1], F32, tag="pv", name=f"pv{j}")
                for j in range(QB // 128)
            ]
        pv = pv_tiles[(h, qb)]
        at = at_tiles.pop((h, k))
        for i in range(KPS):
            kt = g * KPS + i
            for j in range(QB // 128):
                nc.tensor.matmul(
                    pv[j],
                    lhsT=at[:, i, j * 128:(j + 1) * 128],
                    rhs=v_sb[:, h, kt, :],
                    start=(kt == 0),
                    stop=(kt == NKT - 1),
                )

    def epilogue(h, qb):
        pv = pv_tiles.pop((h, qb))
        for j in range(QB // 128):
            rc = rcp.tile([128, 1], F32, tag="rc", name="rc")
            nc.vector.reciprocal(rc, pv[j][:, DH:DH + 1])
            ot = outp.tile([128, DH], BF16, tag="ot", name="ot")
            nc.vector.tensor_scalar_mul(ot, pv[j][:, 0:DH], rc)
            nc.sync.dma_start(
                out[qb * QB + j * 128:qb * QB + (j + 1) * 128, h * DH:(h + 1) * DH],
                ot,
            )

    def attn(h, fills, fill_slots):
        emit_sg(h, 0)
        emit_sg(h, 1)
        fi = 0
        for k in range(NQB * NG):
            if k + 2 < NQB * NG:
                emit_sg(h, k + 2)
            emit_pg(h, k)
            if fi < len(fills) and k >= fill_slots[fi]:
                fills[fi]()
                fi += 1
            if k % NG == NG - 1:
                epilogue(h, k // NG)
        while fi < len(fills):
            fills[fi]()
            fi += 1

    # ---- schedule ----
    # PE warmup (no data dependencies), then pre-phase: K0 | V | Q0
    # interleaved per chunk to match DMA arrival order.
    # Chunks: c0,c1=128tok; c2..c4=256tok; c5,c6=512tok.
    for _ in range(20):
        ps = projp.tile([128, 512], F32, tag="ps", name="ps")
        nc.tensor.matmul(ps, lhsT=dm_a, rhs=dm_b, start=True, stop=True)

    kq_chunk(0, "k", 0)
    kq_chunk(0, "k", 1)
    v_tb(0)
    v_tb(1)
    kq_chunk(0, "k", 2)
    v_tb(2); v_tb(3)
    kq_chunk(0, "q", 0)
    kq_chunk(0, "q", 1)
    kq_chunk(0, "q", 2)
    kq_chunk(0, "k", 3)
    v_tb(4); v_tb(5)
    kq_chunk(0, "q", 3)
    kq_chunk(0, "k", 4)
    v_tb(6); v_tb(7)
    kq_chunk(0, "q", 4)
    kq_chunk(0, "k", 5)
    v_tb(8); v_tb(9); v_tb(10); v_tb(11)
    kq_chunk(0, "k", 6)
    v_tb(12); v_tb(13); v_tb(14); v_tb(15)

    # attn0 fills: Q0 tail (qb4-7), all of K1, Q1 head (tokens 0:512).
    fills0 = [
        lambda: kq_chunk(0, "q", 5),
        lambda: kq_chunk(1, "k", 0),
        lambda: kq_chunk(1, "k", 1),
        lambda: kq_chunk(0, "q", 6),
        lambda: kq_chunk(1, "k", 2),
        lambda: kq_chunk(1, "k", 3),
        lambda: kq_chunk(1, "k", 4),
        lambda: kq_chunk(1, "k", 5),
        lambda: kq_chunk(1, "k", 6),
        lambda: kq_chunk(1, "q", 0),
        lambda: kq_chunk(1, "q", 1),
        lambda: kq_chunk(1, "q", 2),
    ]
    attn(0, fills0, [1, 3, 5, 7, 9, 11, 13, 15, 17, 19, 21, 23])

    # attn1 fills: Q1 tail (tokens 512:2048), needed from qb2 onward.
    fills1 = [
        lambda: kq_chunk(1, "q", 3),
        lambda: kq_chunk(1, "q", 4),
        lambda: kq_chunk(1, "q", 5),
        lambda: kq_chunk(1, "q", 6),
    ]
    attn(1, fills1, [0, 1, 2, 3])


def build_program():
    nc = bacc.Bacc("TRN2", target_bir_lowering=False, debug=False)
    x = nc.dram_tensor("x", [128, KO * SC], BF16, kind="ExternalInput").ap()
    wk0 = nc.dram_tensor("wk0", [128, KO * DH], BF16, kind="ExternalInput").ap()
    wv = nc.dram_tensor("wv", [128, KO * HPC * DH], BF16, kind="ExternalInput").ap()
    wq0 = nc.dram_tensor("wq0", [128, KO * DH], BF16, kind="ExternalInput").ap()
    wk1 = nc.dram_tensor("wk1", [128, KO * DH], BF16, kind="ExternalInput").ap()
    wq1 = nc.dram_tensor("wq1", [128, KO * DH], BF16, kind="ExternalInput").ap()
    bias = nc.dram_tensor("bias", [HPC * DH], F32, kind="ExternalInput").ap()
    out = nc.dram_tensor("out", [SC, HPC * DH], BF16, kind="ExternalOutput").ap()
    with tile.TileContext(nc) as tc:
        with ExitStack() as ctx:
            _mha_body(ctx, tc, out, x, wk0, wv, wq0, wk1, wq1, bias)
    nc.compile()
    return nc


_NC = None


def _get_nc():
    global _NC
    if _NC is None:
        _NC = build_program()
    return _NC


def _pack_w(Wc):
    """[1024 rows, cols] f32 -> [128 ki, KO*cols] bf16, ki-major contiguous."""
    import ml_dtypes

    cols = Wc.shape[1]
    w = Wc.reshape(KO, 128, cols).transpose(1, 0, 2).reshape(128, KO * cols)
    return np.ascontiguousarray(w.astype(ml_dtypes.bfloat16))


def make_in_maps(x, W_qkv, b_qkv):
    import ml_dtypes

    x = np.asarray(x, dtype=np.float32)
    W = np.asarray(W_qkv, dtype=np.float32)
    b = np.asarray(b_qkv, dtype=np.float32)
    x_bf = x.astype(ml_dtypes.bfloat16)
    in_maps = []
    for c in range(N_CORES):
        bsel = c // 4
        h0 = HPC * (c % 4)
        qr = lambda h: np.arange((h0 + h) * DH, (h0 + h + 1) * DH)
        # x^T [1024, 2048] -> [128 ki, KO, tok] -> chunk-packed [128, KO*2048]
        xT = x_bf[bsel].T.reshape(KO, 128, SC).transpose(1, 0, 2)  # [128, KO, 2048]
        xpk = np.concatenate(
            [xT[:, :, off:off + ln].reshape(128, KO * ln) for off, ln in CHUNKS],
            axis=1,
        )
        in_maps.append(
            {
                "x": np.ascontiguousarray(xpk),
                "wk0": _pack_w(W[D + qr(0)].T),
                "wv": _pack_w(W[2 * D:][np.concatenate([qr(0), qr(1)])].T),
                "wq0": _pack_w(W[qr(0)].T),
                "wk1": _pack_w(W[D + qr(1)].T),
                "wq1": _pack_w(W[qr(1)].T),
                "bias": np.ascontiguousarray(np.concatenate([b[qr(0)], b[qr(1)]])),
            }
        )
    return in_maps


def gather_output(results, b_qkv):
    outp = np.empty((B, S, D), np.float32)
    for c in range(N_CORES):
        o = np.asarray(results[c]["out"], dtype=np.float32)
        bsel = c // 4
        h0 = HPC * (c % 4)
        outp[bsel, :, h0 * DH:(h0 + HPC) * DH] = o
    # V bias: out = (P@V)/den + bv since softmax weights sum to 1.
    outp += np.asarray(b_qkv, dtype=np.float32)[2 * D:][None, None, :]
    return outp


def kernel(x, W_qkv, b_qkv, **run_kwargs):
    in_maps = make_in_maps(x, W_qkv, b_qkv)
    res = run_bass_kernel_spmd(_get_nc(), in_maps, core_ids=list(range(N_CORES)), **run_kwargs)
    out = gather_output(res.results, b_qkv)
    if run_kwargs:
        kernel.last_result = res
    return out


# revision 45
# speedup vs baseline: 1.0175x; 1.0031x over previous
"""Multi-head attention Bass kernel for Trainium2, sharded over 8 NeuronCores.

Problem: x [2, 2048, 1024] fp32; W_qkv [3072, 1024]; b_qkv [3072].
  qkv = x @ W_qkv.T + b_qkv ; split into Q,K,V of 8 heads x 128 dims;
  out  = softmax(Q K^T / sqrt(128)) V, heads re-concatenated -> [2, 2048, 1024].

Sharding: 16 (batch, head) pairs over 8 cores -> each core owns one batch
slice (b = core//4) and two heads (h0 = 2*(core%4), h0+1). Each core gets
its batch's x slice [2048, 1024] plus the W^T/bias columns for its heads,
computes the projection and full non-causal attention for its two heads,
and returns [2048, 256] (the two heads' output columns). No collectives.

Kernel structure (per core), tuned to keep the PE continuously busy:
 - K bias is dropped entirely: softmax_k[(Q+bq)·(K+bk)] == softmax_k[(Q+bq)·K]
   since the bk term is constant per query. V bias is added on the host:
   out = (P@V)/den + bv (softmax weights sum to 1).
 - All device inputs are host-packed so every DMA reads a single contiguous
   2-8KB segment per partition (max descriptor efficiency). x chunks are
   ragged (128,128,256,256,256,512,512 tokens) and split across two DGE
   queues (SP + DVE); weight pieces stream on the Act engine's queue in
   first-use order.
 - Projection phase interleaves K0 / V / Q0 per chunk. Q^T/K^T produced
   in [dh, tok] layout directly from the PE; V in [tok, dh] layout with a
   ones column so the P@V matmul also produces softmax denominators.
 - Attention is software-pipelined: score matmuls for group g+2 are
   emitted before the P@V matmuls of group g, so the PE never waits on
   the Act engine's exp. Remaining projection work (Q0 tail, K1, Q1) is
   interleaved as filler so the PE stays saturated while exp runs.
 - Epilogue per 128-token block: reciprocal + scale on DVE, bf16 out.
"""

import math
from contextlib import ExitStack

import numpy as np

import concourse.bass as bass
import concourse.tile as tile
from concourse import bacc, mybir
from concourse.bass_utils import run_bass_kernel_spmd

# Problem constants (hardcoded per the harness contract).
B = 2
S = 2048
D = 1024
H = 8
DH = 128
N_CORES = 8
HPC = 2  # heads per core
SC = S  # tokens per core (one full batch element)
SCALE = 1.0 / math.sqrt(DH)

F32 = mybir.dt.float32
BF16 = mybir.dt.bfloat16

KO = D // 128  # 8 contraction chunks
QB = 256  # query block width
NQB = SC // QB  # 8
NKT = SC // 128  # 16 key tiles
KPS = 4  # key tiles per score group; exp runs on [128, KPS*QB]
NG = NKT // KPS  # 4 score/pv groups per query block

# Ragged token chunks: small ones first so compute starts early.
CH_LENS = [128, 128, 256, 256, 256, 512, 512]
CHUNKS = []
_o = 0
for _ln in CH_LENS:
    CHUNKS.append((_o, _ln))
    _o += _ln
NCH = len(CHUNKS)
# token-block (128 tokens) -> (chunk index, local block index)
TB2C = []
for _ci, (_off, _ln) in enumerate(CHUNKS):
    for _lo in range(_ln // 128):
        TB2C.append((_ci, _lo))


def _mha_body(ctx: ExitStack, tc: tile.TileContext, out, x, wk0, wv, wq0, wk1, wq1, bias):
    nc = tc.nc

    consts = ctx.enter_context(tc.tile_pool(name="consts", bufs=1))
    xtp = ctx.enter_context(tc.tile_pool(name="xtp", bufs=1))
    qkvp = ctx.enter_context(tc.tile_pool(name="qkvp", bufs=1))

    # ---- input DMAs, ordered by first use ----
    # Three DGE queues: SP takes even x chunks; Act interleaves the early
    # weight pieces with odd x chunks; gpsimd (software DGE) takes the
    # late-needed K1/Q1 weights. All reads are contiguous per partition.
    xt = []
    xbase = []
    base = 0
    for c, (off, ln) in enumerate(CHUNKS):
        xt.append(xtp.tile([128, KO, ln], BF16, tag=f"xt{c}", name=f"xt{c}"))
        xbase.append(base)
        base += KO * ln

    def xsrc(c):
        off, ln = CHUNKS[c]
        return x[:, xbase[c]:xbase[c] + KO * ln].rearrange("p (ko t) -> p ko t", ko=KO)

    wtk, wtq = {}, {}
    wtk[0] = consts.tile([128, KO, DH], BF16, tag="wk0", name="wk0")
    wtv = consts.tile([128, KO, HPC * DH], BF16, tag="wv", name="wv")
    wtq[0] = consts.tile([128, KO, DH], BF16, tag="wq0", name="wq0")
    wtk[1] = consts.tile([128, KO, DH], BF16, tag="wk1", name="wk1")
    wtq[1] = consts.tile([128, KO, DH], BF16, tag="wq1", name="wq1")

    # Q biases [128 dh, 1] per head (K bias cancels; V bias added on host).
    bq = []
    for h in range(HPC):
        bt = consts.tile([128, 1], F32, tag=f"bq{h}", name=f"bq{h}")
        bq.append(bt)

    nc.gpsimd.dma_start(bq[0], bias[0:DH].rearrange("(p o) -> p o", o=1))
    nc.gpsimd.dma_start(bq[1], bias[DH:2 * DH].rearrange("(p o) -> p o", o=1))
    nc.sync.dma_start(xt[0], xsrc(0))
    nc.scalar.dma_start(wtk[0], wk0.rearrange("p (ko c) -> p ko c", ko=KO))
    nc.sync.dma_start(xt[1], xsrc(1))
    nc.scalar.dma_start(wtv, wv.rearrange("p (ko c) -> p ko c", ko=KO))
    nc.sync.dma_start(xt[2], xsrc(2))
    nc.scalar.dma_start(xt[3], xsrc(3))
    nc.sync.dma_start(wtq[0], wq0.rearrange("p (ko c) -> p ko c", ko=KO))
    nc.sync.dma_start(xt[4], xsrc(4))
    nc.scalar.dma_start(xt[5], xsrc(5))
    nc.sync.dma_start(xt[6], xsrc(6))
    nc.gpsimd.dma_start(wtk[1], wk1.rearrange("p (ko c) -> p ko c", ko=KO))
    nc.gpsimd.dma_start(wtq[1], wq1.rearrange("p (ko c) -> p ko c", ko=KO))

    # hoist the ACT exp table load to kernel start, under the input DMAs
    warm = consts.tile([128, 1], F32)
    nc.vector.memset(warm, 0.0)
    nc.scalar.activation(warm, warm, mybir.ActivationFunctionType.Exp)

    # PE warmup: throwaway matmuls on memset tiles fill the initial input-DMA
    # wait and ramp the tensor engine's clock before real work arrives.
    dm_a = consts.tile([128, 128], BF16, tag="dm_a", name="dm_a")
    nc.vector.memset(dm_a, 0.0)
    dm_b = consts.tile([128, 512], BF16, tag="dm_b", name="dm_b")
    nc.vector.memset(dm_b, 0.0)

    # ---- persistent QKV tiles ----
    qT = qkvp.tile([128, HPC, SC], BF16, tag="qT")  # [dh, h, tok]
    kT = qkvp.tile([128, HPC, SC], BF16, tag="kT")
    v_sb = qkvp.tile([128, HPC, NKT, DH + 1], BF16, tag="v")  # [tok_i, h, tok_o, dh+1]
    nc.vector.memset(v_sb[:, :, :, DH:DH + 1], 1.0)

    # During the pre-phase the whole of PSUM minus nothing else is live, so
    # the projection pool gets 4 buffers (deeper Tensor->DVE pipelining);
    # it closes before the attention pools (st/pv) open.
    pool_ref = {}
    atp = ctx.enter_context(tc.tile_pool(name="atp", bufs=5))
    outp = ctx.enter_context(tc.tile_pool(name="outp", bufs=3))
    rcp = ctx.enter_context(tc.tile_pool(name="rcp", bufs=4))

    # ---- projection emitters ----
    def kq_chunk(h, kind, c):
        off, ln = CHUNKS[c]
        ps = pool_ref["proj"].tile([128, 512], F32, tag="ps", name="ps")
        w = (wtk if kind == "k" else wtq)[h]
        for ko in range(KO):
            nc.tensor.matmul(
                ps[:, :ln],
                lhsT=w[:, ko, :],
                rhs=xt[c][:, ko, :ln],
                start=(ko == 0),
                stop=(ko == KO - 1),
            )
        if kind == "q":
            nc.vector.tensor_scalar_add(qT[:, h, off:off + ln], ps[:, :ln], bq[h])
        else:
            nc.vector.tensor_copy(kT[:, h, off:off + ln], ps[:, :ln])

    def v_tb(tb):
        c, lo = TB2C[tb]
        ps = pool_ref["proj"].tile([128, 512], F32, tag="ps", name="ps")
        psv = ps[:, :HPC * DH]
        for ko in range(KO):
            nc.tensor.matmul(
                psv,
                lhsT=xt[c][:, ko, lo * 128:(lo + 1) * 128],
                rhs=wtv[:, ko, :],
                start=(ko == 0),
                stop=(ko == KO - 1),
            )
        nc.vector.tensor_copy(
            v_sb[:, :, tb, 0:DH], psv.rearrange("p (h d) -> p h d", h=HPC)
        )

    # ---- attention emitters (software-pipelined) ----
    at_tiles = {}
    pv_tiles = {}

    def emit_sg(h, k):
        qb, g = divmod(k, NG)
        st = pool_ref["st"].tile([128, KPS, QB], F32, tag="st", name="st")
        for i in range(KPS):
            kt = g * KPS + i
            nc.tensor.matmul(
                st[:, i, :],
                lhsT=kT[:, h, kt * 128:(kt + 1) * 128],
                rhs=qT[:, h, qb * QB:(qb + 1) * QB],
                start=True,
                stop=True,
            )
        at = atp.tile([128, KPS, QB], BF16, tag="at", name="at")
        nc.scalar.activation(at, st, mybir.ActivationFunctionType.Exp, scale=SCALE)
        at_tiles[(h, k)] = at

    def emit_pg(h, k):
        qb, g = divmod(k, NG)
        if g == 0:
            # separate tiles so each P@V accumulator gets its own PSUM bank
            pv_tiles[(h, qb)] = [
                pool_ref["pv"].tile([128, DH + 1], F32, tag="pv", name=f"pv{j}")
                for j in range(QB // 128)
            ]
        pv = pv_tiles[(h, qb)]
        at = at_tiles.pop((h, k))
        for i in range(KPS):
            kt = g * KPS + i
            for j in range(QB // 128):
                nc.tensor.matmul(
                    pv[j],
                    lhsT=at[:, i, j * 128:(j + 1) * 128],
                    rhs=v_sb[:, h, kt, :],
                    start=(kt == 0),
                    stop=(kt == NKT - 1),
                )

    def epilogue(h, qb):
        pv = pv_tiles.pop((h, qb))
        for j in range(QB // 128):
            rc = rcp.tile([128, 1], F32, tag="rc", name="rc")
            nc.vector.reciprocal(rc, pv[j][:, DH:DH + 1])
            ot = outp.tile([128, DH], BF16, tag="ot", name="ot")
            nc.vector.tensor_scalar_mul(ot, pv[j][:, 0:DH], rc)
            nc.sync.dma_start(
                out[qb * QB + j * 128:qb * QB + (j + 1) * 128, h * DH:(h + 1) * DH],
                ot,
            )

    def attn(h, fills, fill_slots):
        emit_sg(h, 0)
        emit_sg(h, 1)
        fi = 0
        for k in range(NQB * NG):
            if k + 2 < NQB * NG:
                emit_sg(h, k + 2)
            emit_pg(h, k)
            if fi < len(fills) and k >= fill_slots[fi]:
                fills[fi]()
                fi += 1
            if k % NG == NG - 1:
                epilogue(h, k // NG)
        while fi < len(fills):
            fills[fi]()
            fi += 1

    # ---- schedule ----
    # PE warmup (no data dependencies), then pre-phase: K0 | V | Q0
    # interleaved per chunk to match DMA arrival order.
    # Chunks: c0,c1=128tok; c2..c4=256tok; c5,c6=512tok.
    prep_cm = tc.tile_pool(name="prep", bufs=4, space="PSUM")
    pool_ref["proj"] = prep_cm.__enter__()
    for _ in range(20):
        ps = pool_ref["proj"].tile([128, 512], F32, tag="ps", name="ps")
        nc.tensor.matmul(ps, lhsT=dm_a, rhs=dm_b, start=True, stop=True)

    kq_chunk(0, "k", 0)
    kq_chunk(0, "k", 1)
    v_tb(0)
    v_tb(1)
    kq_chunk(0, "k", 2)
    v_tb(2); v_tb(3)
    kq_chunk(0, "q", 0)
    kq_chunk(0, "q", 1)
    kq_chunk(0, "q", 2)
    kq_chunk(0, "k", 3)
    v_tb(4); v_tb(5)
    kq_chunk(0, "q", 3)
    kq_chunk(0, "k", 4)
    v_tb(6); v_tb(7)
    kq_chunk(0, "q", 4)
    kq_chunk(0, "k", 5)
    v_tb(8); v_tb(9); v_tb(10); v_tb(11)
    kq_chunk(0, "k", 6)
    v_tb(12); v_tb(13); v_tb(14); v_tb(15)

    # Close the wide pre-phase pool, open the attention-era PSUM pools.
    prep_cm.__exit__(None, None, None)
    pool_ref["proj"] = ctx.enter_context(tc.tile_pool(name="projp", bufs=2, space="PSUM"))
    pool_ref["st"] = ctx.enter_context(tc.tile_pool(name="stp", bufs=2, space="PSUM"))
    pool_ref["pv"] = ctx.enter_context(tc.tile_pool(name="pvp", bufs=2, space="PSUM"))

    # attn0 fills: Q0 tail (qb4-7), all of K1, Q1 head (tokens 0:512).
    fills0 = [
        lambda: kq_chunk(0, "q", 5),
        lambda: kq_chunk(1, "k", 0),
        lambda: kq_chunk(1, "k", 1),
        lambda: kq_chunk(0, "q", 6),
        lambda: kq_chunk(1, "k", 2),
        lambda: kq_chunk(1, "k", 3),
        lambda: kq_chunk(1, "k", 4),
        lambda: kq_chunk(1, "k", 5),
        lambda: kq_chunk(1, "k", 6),
        lambda: kq_chunk(1, "q", 0),
        lambda: kq_chunk(1, "q", 1),
        lambda: kq_chunk(1, "q", 2),
    ]
    attn(0, fills0, [1, 3, 5, 7, 9, 11, 13, 15, 17, 19, 21, 23])

    # attn1 fills: Q1 tail (tokens 512:2048), needed from qb2 onward.
    fills1 = [
        lambda: kq_chunk(1, "q", 3),
        lambda: kq_chunk(1, "q", 4),
        lambda: kq_chunk(1, "q", 5),
        lambda: kq_chunk(1, "q", 6),
    ]
    attn(1, fills1, [0, 1, 2, 3])


def build_program():
    nc = bacc.Bacc("TRN2", target_bir_lowering=False, debug=False)
    x = nc.dram_tensor("x", [128, KO * SC], BF16, kind="ExternalInput").ap()
    wk0 = nc.dram_tensor("wk0", [128, KO * DH], BF16, kind="ExternalInput").ap()
    wv = nc.dram_tensor("wv", [128, KO * HPC * DH], BF16, kind="ExternalInput").ap()
    wq0 = nc.dram_tensor("wq0", [128, KO * DH], BF16, kind="ExternalInput").ap()
    wk1 = nc.dram_tensor("wk1", [128, KO * DH], BF16, kind="ExternalInput").ap()
    wq1 = nc.dram_tensor("wq1", [128, KO * DH], BF16, kind="ExternalInput").ap()
    bias = nc.dram_tensor("bias", [HPC * DH], F32, kind="ExternalInput").ap()
    out = nc.dram_tensor("out", [SC, HPC * DH], BF16, kind="ExternalOutput").ap()
    with tile.TileContext(nc) as tc:
        with ExitStack() as ctx:
            _mha_body(ctx, tc, out, x, wk0, wv, wq0, wk1, wq1, bias)
    nc.compile()
    return nc


_NC = None


def _get_nc():
    global _NC
    if _NC is None:
        _NC = build_program()
    return _NC


def _pack_w(Wc):
    """[1024 rows, cols] f32 -> [128 ki, KO*cols] bf16, ki-major contiguous."""
    import ml_dtypes

    cols = Wc.shape[1]
    w = Wc.reshape(KO, 128, cols).transpose(1, 0, 2).reshape(128, KO * cols)
    return np.ascontiguousarray(w.astype(ml_dtypes.bfloat16))


def make_in_maps(x, W_qkv, b_qkv):
    import ml_dtypes

    x = np.asarray(x, dtype=np.float32)
    W = np.asarray(W_qkv, dtype=np.float32)
    b = np.asarray(b_qkv, dtype=np.float32)
    x_bf = x.astype(ml_dtypes.bfloat16)
    in_maps = []
    for c in range(N_CORES):
        bsel = c // 4
        h0 = HPC * (c % 4)
        qr = lambda h: np.arange((h0 + h) * DH, (h0 + h + 1) * DH)
        # x^T [1024, 2048] -> [128 ki, KO, tok] -> chunk-packed [128, KO*2048]
        xT = x_bf[bsel].T.reshape(KO, 128, SC).transpose(1, 0, 2)  # [128, KO, 2048]
        xpk = np.concatenate(
            [xT[:, :, off:off + ln].reshape(128, KO * ln) for off, ln in CHUNKS],
            axis=1,
        )
        in_maps.append(
            {
                "x": np.ascontiguousarray(xpk),
                "wk0": _pack_w(W[D + qr(0)].T),
                "wv": _pack_w(W[2 * D:][np.concatenate([qr(0), qr(1)])].T),
                "wq0": _pack_w(W[qr(0)].T),
                "wk1": _pack_w(W[D + qr(1)].T),
                "wq1": _pack_w(W[qr(1)].T),
                "bias": np.ascontiguousarray(np.concatenate([b[qr(0)], b[qr(1)]])),
            }
        )
    return in_maps


def gather_output(results, b_qkv):
    outp = np.empty((B, S, D), np.float32)
    for c in range(N_CORES):
        o = np.asarray(results[c]["out"], dtype=np.float32)
        bsel = c // 4
        h0 = HPC * (c % 4)
        outp[bsel, :, h0 * DH:(h0 + HPC) * DH] = o
    # V bias: out = (P@V)/den + bv since softmax weights sum to 1.
    outp += np.asarray(b_qkv, dtype=np.float32)[2 * D:][None, None, :]
    return outp


def kernel(x, W_qkv, b_qkv, **run_kwargs):
    in_maps = make_in_maps(x, W_qkv, b_qkv)
    res = run_bass_kernel_spmd(_get_nc(), in_maps, core_ids=list(range(N_CORES)), **run_kwargs)
    out = gather_output(res.results, b_qkv)
    if run_kwargs:
        kernel.last_result = res
    return out


# revision 46
# speedup vs baseline: 1.0253x; 1.0077x over previous
"""Multi-head attention Bass kernel for Trainium2, sharded over 8 NeuronCores.

Problem: x [2, 2048, 1024] fp32; W_qkv [3072, 1024]; b_qkv [3072].
  qkv = x @ W_qkv.T + b_qkv ; split into Q,K,V of 8 heads x 128 dims;
  out  = softmax(Q K^T / sqrt(128)) V, heads re-concatenated -> [2, 2048, 1024].

Sharding: 16 (batch, head) pairs over 8 cores -> each core owns one batch
slice (b = core//4) and two heads (h0 = 2*(core%4), h0+1). Each core gets
its batch's x slice [2048, 1024] plus the W^T/bias columns for its heads,
computes the projection and full non-causal attention for its two heads,
and returns [2048, 256] (the two heads' output columns). No collectives.

Kernel structure (per core), tuned to keep the PE continuously busy:
 - K bias is dropped entirely: softmax_k[(Q+bq)·(K+bk)] == softmax_k[(Q+bq)·K]
   since the bk term is constant per query. V bias is added on the host:
   out = (P@V)/den + bv (softmax weights sum to 1).
 - All device inputs are host-packed so every DMA reads a single contiguous
   2-8KB segment per partition (max descriptor efficiency). x chunks are
   ragged (128,128,256,256,256,512,512 tokens) and split across two DGE
   queues (SP + DVE); weight pieces stream on the Act engine's queue in
   first-use order.
 - Projection phase interleaves K0 / V / Q0 per chunk. Q^T/K^T produced
   in [dh, tok] layout directly from the PE; V in [tok, dh] layout with a
   ones column so the P@V matmul also produces softmax denominators.
 - Attention is software-pipelined: score matmuls for group g+2 are
   emitted before the P@V matmuls of group g, so the PE never waits on
   the Act engine's exp. Remaining projection work (Q0 tail, K1, Q1) is
   interleaved as filler so the PE stays saturated while exp runs.
 - Epilogue per 128-token block: reciprocal + scale on DVE, bf16 out.
"""

import math
from contextlib import ExitStack

import numpy as np

import concourse.bass as bass
import concourse.tile as tile
from concourse import bacc, mybir
from concourse.bass_utils import run_bass_kernel_spmd

# Problem constants (hardcoded per the harness contract).
B = 2
S = 2048
D = 1024
H = 8
DH = 128
N_CORES = 8
HPC = 2  # heads per core
SC = S  # tokens per core (one full batch element)
SCALE = 1.0 / math.sqrt(DH)

F32 = mybir.dt.float32
BF16 = mybir.dt.bfloat16

KO = D // 128  # 8 contraction chunks
QB = 256  # query block width
NQB = SC // QB  # 8
NKT = SC // 128  # 16 key tiles
KPS = 4  # key tiles per score group; exp runs on [128, KPS*QB]
NG = NKT // KPS  # 4 score/pv groups per query block

# Ragged token chunks: small ones first so compute starts early.
CH_LENS = [128, 128, 256, 256, 256, 512, 512]
CHUNKS = []
_o = 0
for _ln in CH_LENS:
    CHUNKS.append((_o, _ln))
    _o += _ln
NCH = len(CHUNKS)
# token-block (128 tokens) -> (chunk index, local block index)
TB2C = []
for _ci, (_off, _ln) in enumerate(CHUNKS):
    for _lo in range(_ln // 128):
        TB2C.append((_ci, _lo))


def _mha_body(ctx: ExitStack, tc: tile.TileContext, out, x, wk0, wv, wq0, wk1, wq1, bias):
    nc = tc.nc

    consts = ctx.enter_context(tc.tile_pool(name="consts", bufs=1))
    xtp = ctx.enter_context(tc.tile_pool(name="xtp", bufs=1))
    qkvp = ctx.enter_context(tc.tile_pool(name="qkvp", bufs=1))

    # ---- input DMAs, ordered by first use ----
    # Three DGE queues: SP takes even x chunks; Act interleaves the early
    # weight pieces with odd x chunks; gpsimd (software DGE) takes the
    # late-needed K1/Q1 weights. All reads are contiguous per partition.
    xt = []
    xbase = []
    base = 0
    for c, (off, ln) in enumerate(CHUNKS):
        xt.append(xtp.tile([128, KO, ln], BF16, tag=f"xt{c}", name=f"xt{c}"))
        xbase.append(base)
        base += KO * ln

    def xsrc(c):
        off, ln = CHUNKS[c]
        return x[:, xbase[c]:xbase[c] + KO * ln].rearrange("p (ko t) -> p ko t", ko=KO)

    wtk, wtq = {}, {}
    wtk[0] = consts.tile([128, KO, DH], BF16, tag="wk0", name="wk0")
    wtv = consts.tile([128, KO, HPC * DH], BF16, tag="wv", name="wv")
    wtq[0] = consts.tile([128, KO, DH], BF16, tag="wq0", name="wq0")
    wtk[1] = consts.tile([128, KO, DH], BF16, tag="wk1", name="wk1")
    wtq[1] = consts.tile([128, KO, DH], BF16, tag="wq1", name="wq1")

    # Q biases [128 dh, 1] per head (K bias cancels; V bias added on host).
    bq = []
    for h in range(HPC):
        bt = consts.tile([128, 1], F32, tag=f"bq{h}", name=f"bq{h}")
        bq.append(bt)

    nc.gpsimd.dma_start(bq[0], bias[0:DH].rearrange("(p o) -> p o", o=1))
    nc.gpsimd.dma_start(bq[1], bias[DH:2 * DH].rearrange("(p o) -> p o", o=1))
    nc.sync.dma_start(xt[0], xsrc(0))
    nc.scalar.dma_start(wtk[0], wk0.rearrange("p (ko c) -> p ko c", ko=KO))
    nc.sync.dma_start(xt[1], xsrc(1))
    nc.scalar.dma_start(wtv, wv.rearrange("p (ko c) -> p ko c", ko=KO))
    nc.sync.dma_start(xt[2], xsrc(2))
    nc.scalar.dma_start(xt[3], xsrc(3))
    nc.sync.dma_start(wtq[0], wq0.rearrange("p (ko c) -> p ko c", ko=KO))
    nc.sync.dma_start(xt[4], xsrc(4))
    nc.scalar.dma_start(xt[5], xsrc(5))
    nc.sync.dma_start(xt[6], xsrc(6))
    nc.gpsimd.dma_start(wtk[1], wk1.rearrange("p (ko c) -> p ko c", ko=KO))
    nc.gpsimd.dma_start(wtq[1], wq1.rearrange("p (ko c) -> p ko c", ko=KO))

    # PE warmup: throwaway matmuls on memset tiles fill the initial input-DMA
    # wait and ramp the tensor engine's clock before real work arrives.
    dm_a = consts.tile([128, 128], BF16, tag="dm_a", name="dm_a")
    nc.vector.memset(dm_a, 0.0)
    dm_b = consts.tile([128, 512], BF16, tag="dm_b", name="dm_b")
    nc.vector.memset(dm_b, 0.0)

    # hoist the ACT exp table load to kernel start, under the input DMAs
    warm = consts.tile([128, 1], F32)
    nc.vector.memset(warm, 0.0)
    nc.scalar.activation(warm, warm, mybir.ActivationFunctionType.Exp)

    # ---- persistent QKV tiles ----
    qT = qkvp.tile([128, HPC, SC], BF16, tag="qT")  # [dh, h, tok]
    kT = qkvp.tile([128, HPC, SC], BF16, tag="kT")
    v_sb = qkvp.tile([128, HPC, NKT, DH + 1], BF16, tag="v")  # [tok_i, h, tok_o, dh+1]
    nc.vector.memset(v_sb[:, :, :, DH:DH + 1], 1.0)

    # During the pre-phase the whole of PSUM minus nothing else is live, so
    # the projection pool gets 4 buffers (deeper Tensor->DVE pipelining);
    # it closes before the attention pools (st/pv) open.
    pool_ref = {}
    atp = ctx.enter_context(tc.tile_pool(name="atp", bufs=5))
    outp = ctx.enter_context(tc.tile_pool(name="outp", bufs=3))
    rcp = ctx.enter_context(tc.tile_pool(name="rcp", bufs=4))

    # ---- projection emitters ----
    def kq_chunk(h, kind, c):
        off, ln = CHUNKS[c]
        ps = pool_ref["proj"].tile([128, 512], F32, tag="ps", name="ps")
        w = (wtk if kind == "k" else wtq)[h]
        for ko in range(KO):
            nc.tensor.matmul(
                ps[:, :ln],
                lhsT=w[:, ko, :],
                rhs=xt[c][:, ko, :ln],
                start=(ko == 0),
                stop=(ko == KO - 1),
            )
        if kind == "q":
            nc.vector.tensor_scalar_add(qT[:, h, off:off + ln], ps[:, :ln], bq[h])
        else:
            nc.vector.tensor_copy(kT[:, h, off:off + ln], ps[:, :ln])

    def v_tb(tb):
        c, lo = TB2C[tb]
        ps = pool_ref["proj"].tile([128, 512], F32, tag="ps", name="ps")
        psv = ps[:, :HPC * DH]
        for ko in range(KO):
            nc.tensor.matmul(
                psv,
                lhsT=xt[c][:, ko, lo * 128:(lo + 1) * 128],
                rhs=wtv[:, ko, :],
                start=(ko == 0),
                stop=(ko == KO - 1),
            )
        nc.vector.tensor_copy(
            v_sb[:, :, tb, 0:DH], psv.rearrange("p (h d) -> p h d", h=HPC)
        )

    # ---- attention emitters (software-pipelined) ----
    at_tiles = {}
    pv_tiles = {}

    def emit_sg(h, k):
        qb, g = divmod(k, NG)
        st = pool_ref["st"].tile([128, KPS, QB], F32, tag="st", name="st")
        for i in range(KPS):
            kt = g * KPS + i
            nc.tensor.matmul(
                st[:, i, :],
                lhsT=kT[:, h, kt * 128:(kt + 1) * 128],
                rhs=qT[:, h, qb * QB:(qb + 1) * QB],
                start=True,
                stop=True,
            )
        at = atp.tile([128, KPS, QB], BF16, tag="at", name="at")
        nc.scalar.activation(at, st, mybir.ActivationFunctionType.Exp, scale=SCALE)
        at_tiles[(h, k)] = at

    def emit_pg(h, k):
        qb, g = divmod(k, NG)
        if g == 0:
            # separate tiles so each P@V accumulator gets its own PSUM bank
            pv_tiles[(h, qb)] = [
                pool_ref["pv"].tile([128, DH + 1], F32, tag="pv", name=f"pv{j}")
                for j in range(QB // 128)
            ]
        pv = pv_tiles[(h, qb)]
        at = at_tiles.pop((h, k))
        for i in range(KPS):
            kt = g * KPS + i
            for j in range(QB // 128):
                nc.tensor.matmul(
                    pv[j],
                    lhsT=at[:, i, j * 128:(j + 1) * 128],
                    rhs=v_sb[:, h, kt, :],
                    start=(kt == 0),
                    stop=(kt == NKT - 1),
                )

    def epilogue(h, qb):
        pv = pv_tiles.pop((h, qb))
        for j in range(QB // 128):
            rc = rcp.tile([128, 1], F32, tag="rc", name="rc")
            nc.vector.reciprocal(rc, pv[j][:, DH:DH + 1])
            ot = outp.tile([128, DH], BF16, tag="ot", name="ot")
            nc.vector.tensor_scalar_mul(ot, pv[j][:, 0:DH], rc)
            nc.sync.dma_start(
                out[qb * QB + j * 128:qb * QB + (j + 1) * 128, h * DH:(h + 1) * DH],
                ot,
            )

    def attn(h, fills, fill_slots):
        emit_sg(h, 0)
        emit_sg(h, 1)
        fi = 0
        for k in range(NQB * NG):
            if k + 2 < NQB * NG:
                emit_sg(h, k + 2)
            emit_pg(h, k)
            if fi < len(fills) and k >= fill_slots[fi]:
                fills[fi]()
                fi += 1
            if k % NG == NG - 1:
                epilogue(h, k // NG)
        while fi < len(fills):
            fills[fi]()
            fi += 1

    # ---- schedule ----
    # PE warmup (no data dependencies), then pre-phase: K0 | V | Q0
    # interleaved per chunk to match DMA arrival order.
    # Chunks: c0,c1=128tok; c2..c4=256tok; c5,c6=512tok.
    prep_cm = tc.tile_pool(name="prep", bufs=4, space="PSUM")
    pool_ref["proj"] = prep_cm.__enter__()
    for _ in range(20):
        ps = pool_ref["proj"].tile([128, 512], F32, tag="ps", name="ps")
        nc.tensor.matmul(ps, lhsT=dm_a, rhs=dm_b, start=True, stop=True)

    kq_chunk(0, "k", 0)
    kq_chunk(0, "k", 1)
    v_tb(0)
    v_tb(1)
    kq_chunk(0, "k", 2)
    v_tb(2); v_tb(3)
    kq_chunk(0, "q", 0)
    kq_chunk(0, "q", 1)
    kq_chunk(0, "q", 2)
    kq_chunk(0, "k", 3)
    v_tb(4); v_tb(5)
    kq_chunk(0, "q", 3)
    kq_chunk(0, "k", 4)
    v_tb(6); v_tb(7)
    kq_chunk(0, "q", 4)
    kq_chunk(0, "k", 5)
    v_tb(8); v_tb(9); v_tb(10); v_tb(11)
    kq_chunk(0, "k", 6)
    v_tb(12); v_tb(13); v_tb(14); v_tb(15)

    # Close the wide pre-phase pool, open the attention-era PSUM pools.
    prep_cm.__exit__(None, None, None)
    pool_ref["proj"] = ctx.enter_context(tc.tile_pool(name="projp", bufs=2, space="PSUM"))
    pool_ref["st"] = ctx.enter_context(tc.tile_pool(name="stp", bufs=2, space="PSUM"))
    pool_ref["pv"] = ctx.enter_context(tc.tile_pool(name="pvp", bufs=2, space="PSUM"))

    # attn0 fills: Q0 tail (qb4-7), all of K1, Q1 head (tokens 0:512).
    fills0 = [
        lambda: kq_chunk(0, "q", 5),
        lambda: kq_chunk(1, "k", 0),
        lambda: kq_chunk(1, "k", 1),
        lambda: kq_chunk(0, "q", 6),
        lambda: kq_chunk(1, "k", 2),
        lambda: kq_chunk(1, "k", 3),
        lambda: kq_chunk(1, "k", 4),
        lambda: kq_chunk(1, "k", 5),
        lambda: kq_chunk(1, "k", 6),
        lambda: kq_chunk(1, "q", 0),
        lambda: kq_chunk(1, "q", 1),
        lambda: kq_chunk(1, "q", 2),
    ]
    attn(0, fills0, [1, 3, 5, 7, 9, 11, 13, 15, 17, 19, 21, 23])

    # attn1 fills: Q1 tail (tokens 512:2048), needed from qb2 onward.
    fills1 = [
        lambda: kq_chunk(1, "q", 3),
        lambda: kq_chunk(1, "q", 4),
        lambda: kq_chunk(1, "q", 5),
        lambda: kq_chunk(1, "q", 6),
    ]
    attn(1, fills1, [0, 1, 2, 3])


def build_program():
    nc = bacc.Bacc("TRN2", target_bir_lowering=False, debug=False)
    x = nc.dram_tensor("x", [128, KO * SC], BF16, kind="ExternalInput").ap()
    wk0 = nc.dram_tensor("wk0", [128, KO * DH], BF16, kind="ExternalInput").ap()
    wv = nc.dram_tensor("wv", [128, KO * HPC * DH], BF16, kind="ExternalInput").ap()
    wq0 = nc.dram_tensor("wq0", [128, KO * DH], BF16, kind="ExternalInput").ap()
    wk1 = nc.dram_tensor("wk1", [128, KO * DH], BF16, kind="ExternalInput").ap()
    wq1 = nc.dram_tensor("wq1", [128, KO * DH], BF16, kind="ExternalInput").ap()
    bias = nc.dram_tensor("bias", [HPC * DH], F32, kind="ExternalInput").ap()
    out = nc.dram_tensor("out", [SC, HPC * DH], BF16, kind="ExternalOutput").ap()
    with tile.TileContext(nc) as tc:
        with ExitStack() as ctx:
            _mha_body(ctx, tc, out, x, wk0, wv, wq0, wk1, wq1, bias)
    nc.compile()
    return nc


_NC = None


def _get_nc():
    global _NC
    if _NC is None:
        _NC = build_program()
    return _NC


def _pack_w(Wc):
    """[1024 rows, cols] f32 -> [128 ki, KO*cols] bf16, ki-major contiguous."""
    import ml_dtypes

    cols = Wc.shape[1]
    w = Wc.reshape(KO, 128, cols).transpose(1, 0, 2).reshape(128, KO * cols)
    return np.ascontiguousarray(w.astype(ml_dtypes.bfloat16))


def make_in_maps(x, W_qkv, b_qkv):
    import ml_dtypes

    x = np.asarray(x, dtype=np.float32)
    W = np.asarray(W_qkv, dtype=np.float32)
    b = np.asarray(b_qkv, dtype=np.float32)
    x_bf = x.astype(ml_dtypes.bfloat16)
    in_maps = []
    for c in range(N_CORES):
        bsel = c // 4
        h0 = HPC * (c % 4)
        qr = lambda h: np.arange((h0 + h) * DH, (h0 + h + 1) * DH)
        # x^T [1024, 2048] -> [128 ki, KO, tok] -> chunk-packed [128, KO*2048]
        xT = x_bf[bsel].T.reshape(KO, 128, SC).transpose(1, 0, 2)  # [128, KO, 2048]
        xpk = np.concatenate(
            [xT[:, :, off:off + ln].reshape(128, KO * ln) for off, ln in CHUNKS],
            axis=1,
        )
        in_maps.append(
            {
                "x": np.ascontiguousarray(xpk),
                "wk0": _pack_w(W[D + qr(0)].T),
                "wv": _pack_w(W[2 * D:][np.concatenate([qr(0), qr(1)])].T),
                "wq0": _pack_w(W[qr(0)].T),
                "wk1": _pack_w(W[D + qr(1)].T),
                "wq1": _pack_w(W[qr(1)].T),
                "bias": np.ascontiguousarray(np.concatenate([b[qr(0)], b[qr(1)]])),
            }
        )
    return in_maps


def gather_output(results, b_qkv):
    outp = np.empty((B, S, D), np.float32)
    for c in range(N_CORES):
        o = np.asarray(results[c]["out"], dtype=np.float32)
        bsel = c // 4
        h0 = HPC * (c % 4)
        outp[bsel, :, h0 * DH:(h0 + HPC) * DH] = o
    # V bias: out = (P@V)/den + bv since softmax weights sum to 1.
    outp += np.asarray(b_qkv, dtype=np.float32)[2 * D:][None, None, :]
    return outp


def kernel(x, W_qkv, b_qkv, **run_kwargs):
    in_maps = make_in_maps(x, W_qkv, b_qkv)
    res = run_bass_kernel_spmd(_get_nc(), in_maps, core_ids=list(range(N_CORES)), **run_kwargs)
    out = gather_output(res.results, b_qkv)
    if run_kwargs:
        kernel.last_result = res
    return out
